# revision 58
# baseline (speedup 1.0000x reference)
"""Multi-head attention (B=4, N=2048, C=1024, H=16, D=64) on 8 TRN2 cores.

Sharding: core c -> batch b = c%4, head-group g = c//4 (local heads 0..7 are
global heads 8g..8g+7).  Each core computes its head group's contribution to
the output projection for its batch; host sums core b + core b+4 and adds
const_row = qkv_b[2048:] @ proj_w + proj_b (V-bias folds exactly through the
row-normalized attention: attn @ (1*bv^T) = 1*bv^T).

v3: all-bf16 datapath; one seamless software pipeline over every
(head-pair pr, query-block qb, key-block-pair tg) slot so the scores ->
exp(scalar engine) stream NEVER pauses at qb/pr boundaries (the exp stream is
the co-bottleneck: ~285us of scalar-engine work vs ~330us of matmul).  Each
slot emits: scores(g) [2 key blocks x 2 heads, row-packed K=64 matmuls],
PV(g-2) [lag-2 so exp latency is off the critical path], exp(g), plus paced
filler work (QKV projection for later head-pairs, output projection) to keep
the tensor engine ahead of the scalar engine.  Softmax normalization is
dripped through deferred DVE reciprocal halves + DRAM-bounce broadcast +
GPSIMD multiply, all off the matmul critical path.

Per-core device layout:
  x_sb[j]  [128, 2048] bf16   xT rows j*128.. (C x N), j = 0..7
  wq/wk/wv_sb[j] [128, 512]   W columns for this head group, per input-row j
  QT/KT[pr] [128, 2048] bf16  partition = dim-in-pair (2 heads x 64), free = tok
  VT[pr]   [128, 16, 2, 65]   partition = token-in-block; col 64 = ones (denom)
  OT[pr]   [128, 2048] bf16   normalized attention output, chan-pair x token
  out      [2048, 1024] bf16  partial projection output
"""

import functools
import sys

sys.path.insert(0, "/opt/trn_rl_repo")

from collections import deque
from contextlib import ExitStack

import numpy as np
from ml_dtypes import bfloat16

from concourse import bacc, mybir, tile
from concourse.bass_utils import run_bass_kernel_spmd

F32 = mybir.dt.float32
BF16 = mybir.dt.bfloat16
EXP = mybir.ActivationFunctionType.Exp
LN = mybir.ActivationFunctionType.Ln
CPY = mybir.ActivationFunctionType.Copy
ADD = mybir.AluOpType.add
MULT = mybir.AluOpType.mult

B, N, C, H, D = 4, 2048, 1024, 16, 64
SCALE = 0.125


def _build(dbg=False):
    nc = bacc.Bacc("TRN2", target_bir_lowering=False, debug=False)
    xT = nc.dram_tensor("xT", [1024, 2048], BF16, kind="ExternalInput").ap()
    wcat = nc.dram_tensor("wcat", [1024, 1536], BF16, kind="ExternalInput").ap()
    qbias = nc.dram_tensor("qb", [128, 4], F32, kind="ExternalInput").ap()
    kbias = nc.dram_tensor("kb", [128, 4], F32, kind="ExternalInput").ap()
    pw = nc.dram_tensor("pw", [512, 1024], BF16, kind="ExternalInput").ap()
    out = nc.dram_tensor("out", [2048, 1024], BF16, kind="ExternalOutput").ap()
    scratch = nc.dram_tensor("scratch", [32, 512], F32).ap()
    if dbg:
        dbgb = nc.dram_tensor("dbgb", [8, 2048], BF16,
                              kind="ExternalOutput").ap()
        dbgf = nc.dram_tensor("dbgf", [66, 512], F32,
                              kind="ExternalOutput").ap()

    with tile.TileContext(nc) as tc, ExitStack() as ctx:
        sb = ctx.enter_context(tc.tile_pool(name="sb", bufs=1))
        ps = ctx.enter_context(tc.tile_pool(name="ps", bufs=1, space="PSUM"))

        x_sb = [sb.tile([128, 2048], BF16, name=f"x{j}") for j in range(8)]
        wq_sb = [sb.tile([128, 512], BF16, name=f"wq{j}") for j in range(8)]
        wk_sb = [sb.tile([128, 512], BF16, name=f"wk{j}") for j in range(8)]
        wv_sb = [sb.tile([128, 512], BF16, name=f"wv{j}") for j in range(8)]
        pw_sb = sb.tile([128, 4, 1024], BF16, tag="pw")
        QT = [[sb.tile([128, 1024], BF16, name=f"QT{p}h{h}") for h in (0, 1)]
              for p in range(4)]
        KT = [sb.tile([128, 2048], BF16, name=f"KT{p}") for p in range(4)]
        VT = [[sb.tile([128, 8, 2, 65], BF16, name=f"VT{p}h{h}") for h in (0, 1)]
              for p in range(4)]
        OT = [sb.tile([128, 2048], BF16, name=f"OT{p}") for p in range(4)]
        qb_sb = sb.tile([128, 4], F32, tag="qb")
        kb_sb = sb.tile([128, 4], F32, tag="kb")
        zc = sb.tile([128, 8, 2, 1], F32, tag="zc")
        onec = sb.tile([128, 1], F32, tag="onec")

        for j in range(8):
            nc.sync.dma_start(wq_sb[j][:], wcat[j * 128:(j + 1) * 128, 0:512])
            eng = nc.sync if j % 2 == 0 else nc.scalar
            eng.dma_start(x_sb[j][:], xT[j * 128:(j + 1) * 128, :])
        nc.sync.dma_start(qb_sb[:], qbias[:])
        nc.sync.dma_start(kb_sb[:], kbias[:])
        for j in range(8):
            nc.sync.dma_start(wk_sb[j][:], wcat[j * 128:(j + 1) * 128, 512:1024])
        for j in range(8):
            nc.sync.dma_start(wv_sb[j][:], wcat[j * 128:(j + 1) * 128, 1024:1536])
        nc.vector.memset(zc[:], 0.0)
        nc.vector.memset(onec[:], 1.0)
        for pr in range(4):
            nc.sync.dma_start(pw_sb[:, pr, :], pw[pr * 128:(pr + 1) * 128, :])
            for h in (0, 1):
                nc.vector.tensor_scalar(out=VT[pr][h][:, :, :, 64:65],
                                        in0=zc[:], scalar1=onec[:],
                                        scalar2=None, op0=ADD)

        def qkv_groups(pr, rings=("acc",)):
            """32 emitters: (owner_pr, pe_cycles, fn).  For fills (pr>0), K
            comes first (the next head-pair's first scores need the whole K
            panel the moment its attention starts), then Q for query-block 0,
            then V, then the remaining Q blocks."""
            ppr = max(pr, 0)
            rr = {"i": 0}

            def ring():
                rr["i"] += 1
                return rings[rr["i"] % len(rings)]

            def gq(nb):
                done.add(("q", ppr, nb))
                acc = ps.tile([128, 512], F32, tag=ring(), bufs=2, name="acc")
                for j in range(8):
                    nc.tensor.matmul(acc[:, 0:256],
                                     wq_sb[j][:, ppr * 128:(ppr + 1) * 128],
                                     x_sb[j][:, nb * 256:(nb + 1) * 256],
                                     start=(j == 0), stop=(j == 7))
                nc.vector.tensor_scalar(
                    out=QT[ppr][nb // 4][:, (nb % 4) * 256:(nb % 4 + 1) * 256],
                    in0=acc[:, 0:256],
                    scalar1=qb_sb[:, ppr:ppr + 1], scalar2=None, op0=ADD)

            def gk(nb):
                done.add(("k", ppr, nb))
                acc = ps.tile([128, 512], F32, tag=ring(), bufs=2, name="acc")
                for j in range(8):
                    nc.tensor.matmul(acc[:, 0:256],
                                     wk_sb[j][:, ppr * 128:(ppr + 1) * 128],
                                     x_sb[j][:, nb * 256:(nb + 1) * 256],
                                     start=(j == 0), stop=(j == 7))
                nc.vector.tensor_scalar(
                    out=KT[ppr][:, nb * 256:(nb + 1) * 256], in0=acc[:, 0:256],
                    scalar1=kb_sb[:, ppr:ppr + 1], scalar2=None, op0=ADD)

            def gv(t):
                done.add(("v", ppr, t))
                acc = ps.tile([128, 512], F32, tag=ring(), bufs=2, name="acc")
                for j in range(8):
                    nc.tensor.matmul(acc[:, 0:128],
                                     x_sb[j][:, t * 128:(t + 1) * 128],
                                     wv_sb[j][:, ppr * 128:(ppr + 1) * 128],
                                     start=(j == 0), stop=(j == 7))
                nc.vector.tensor_copy(
                    out=VT[ppr][t // 8][:, t % 8, :, 0:64],
                    in_=acc[:, 0:128].rearrange("p (h d) -> p h d", h=2))

            if pr == 0:
                # serial prologue: just enough for attention to start
                # (K panel, first Q half-panel, first V half-panel); the
                # rest streams in as paced fills
                for nb in range(4):
                    yield pr, 2048, functools.partial(gq, nb)
                for nb in range(8):
                    yield pr, 2048, functools.partial(gk, nb)
                for t in range(8):
                    yield pr, 1024, functools.partial(gv, t)
            elif pr == -1:
                # pr0 leftovers, deadline-ordered for the early slots
                for t in range(8, 16):
                    yield 0, 1024, functools.partial(gv, t)
                for nb in range(4, 8):
                    yield 0, 2048, functools.partial(gq, nb)
            else:
                for nb in range(8):
                    yield pr, 2048, functools.partial(gk, nb)
                for nb in range(4):
                    yield pr, 2048, functools.partial(gq, nb)
                for t in range(8):
                    yield pr, 1024, functools.partial(gv, t)
                for nb in range(4, 8):
                    yield pr, 2048, functools.partial(gq, nb)
                for t in range(8, 16):
                    yield pr, 1024, functools.partial(gv, t)

        def proj_groups(qb, tail=False):
            """8 emitters: output projection for query block qb.  In the tail
            (after the last exp) the scalar engine is idle: its Copy drains
            PSUM and both the acc and oaug rings are free; output DMAs split
            across two engine queues."""
            q0 = qb * 512
            for gi, (ns, co) in enumerate((n, c) for n in range(4)
                                          for c in range(2)):
                def gp(ns=ns, co=co, gi=gi):
                    ring = ("oaug" if (tail and gi % 2) else "acc")
                    pj = ps.tile([128, 512], F32, tag=ring, bufs=2, name="pj")
                    for pr4 in range(4):
                        nc.tensor.matmul(
                            pj[:],
                            OT[pr4][:, q0 + ns * 128:q0 + (ns + 1) * 128],
                            pw_sb[:, pr4, co * 512:(co + 1) * 512],
                            start=(pr4 == 0), stop=(pr4 == 3))
                    so = sb.tile([128, 512], BF16, tag="so", bufs=4, name="so")
                    # alternate the PSUM-draining copy between the scalar
                    # engine (which has slack during head-pair 3) and the DVE
                    if gi % 2:
                        nc.scalar.activation(so[:], pj[:], CPY,
                                             bias=0.0, scale=1.0)
                    else:
                        nc.vector.tensor_copy(out=so[:], in_=pj[:])
                    eng = nc.scalar if (tail and gi % 2) else nc.sync
                    eng.dma_start(
                        out[q0 + ns * 128:q0 + (ns + 1) * 128,
                            co * 512:(co + 1) * 512], so[:])
                yield 9, 2048, gp

        done = set()

        # ---- paced fill + deferred-normalize machinery -------------------
        fills = deque()
        pending = deque()
        st8 = {"filled": 0, "target": 0}

        def pace():
            st8["target"] += 2200
            popped = 0
            while fills and st8["filled"] < st8["target"] and popped < 3:
                _, cyc, g = fills.popleft()
                g()
                st8["filled"] += cyc
                popped += 1

        def flush(owner_max):
            while fills and fills[0][0] <= owner_max:
                _, cyc, g = fills.popleft()
                g()
                st8["filled"] += cyc

        def need(kind, pr, idx):
            # just-in-time: pop fills (they are deadline-ordered) until the
            # required QKV panel write has been emitted
            while (kind, pr, idx) not in done and fills:
                _, cyc, g = fills.popleft()
                g()
                st8["filled"] += cyc

        def emit_norm(pr, qb, oaug0, oaug1):
            """Free the PV accumulators via immediate SBUF copies, then defer
            [reciprocal halves -> DRAM-bounce broadcast -> GPSIMD multiply]
            into later pipeline slots.  The last block instead runs its
            reciprocal as exp(-ln d) on the then-idle scalar engine."""
            final = pr == 3 and qb == 3
            q0 = qb * 512
            osts, rcs, rbs = [], [], []
            for hh, oaug in ((0, oaug0), (1, oaug1)):
                ost = sb.tile([65, 512], F32, tag="ost", bufs=4, name="ost")
                nc.vector.tensor_copy(out=ost[:], in_=oaug[:])
                if dbg and pr == 0 and qb == 0 and hh == 0:
                    nc.sync.dma_start(dbgf[0:65, :], ost[:])
                osts.append(ost)
                rcs.append(sb.tile([1, 512], F32, tag="rc", bufs=4, name="rc"))
                rbs.append(sb.tile([64, 512], F32, tag="rb", bufs=4, name="rb"))

            def recip_piece(hh, half):
                row = pr * 8 + qb * 2 + hh
                lo, hi = half * 256, (half + 1) * 256
                nc.vector.reciprocal(rcs[hh][0:1, lo:hi], osts[hh][64:65, lo:hi])
                nc.sync.dma_start(scratch[row:row + 1, lo:hi],
                                  rcs[hh][0:1, lo:hi])

            def recip_act(hh):
                row = pr * 8 + qb * 2 + hh
                lns = sb.tile([1, 512], F32, tag="lns", bufs=2, name="lns")
                nc.scalar.activation(lns[:], osts[hh][64:65, :], LN,
                                     bias=0.0, scale=1.0)
                nc.scalar.activation(rcs[hh][0:1, :], lns[:], EXP,
                                     bias=0.0, scale=-1.0)
                nc.sync.dma_start(scratch[row:row + 1, :], rcs[hh][0:1, :])

            def bcast_piece():
                for hh in range(2):
                    row = pr * 8 + qb * 2 + hh
                    nc.sync.dma_start(
                        rbs[hh][:],
                        scratch[row:row + 1, :].to_broadcast((64, 512)))

            def norm_piece(hh):
                nc.gpsimd.tensor_tensor(out=OT[pr][hh * 64:(hh + 1) * 64,
                                                   q0:q0 + 512],
                                        in0=osts[hh][0:64, :], in1=rbs[hh][:],
                                        op=MULT)

            if final:
                # all exps are done: run 1/d = exp(-ln d) on the idle scalar
                # engine inline
                recip_act(0)
                recip_act(1)
                bcast_piece()
                norm_piece(0)
                norm_piece(1)
            else:
                pending.extend([
                    lambda: recip_piece(0, 0), lambda: recip_piece(1, 0),
                    lambda: recip_piece(0, 1), lambda: recip_piece(1, 1),
                    bcast_piece,
                    lambda: norm_piece(0), lambda: norm_piece(1),
                ])

        # ---- prologue: QKV for head-pair 0 -------------------------------
        # Spread the partial j-accumulations over three PSUM rings so up to
        # six groups are in flight while the x tiles stream in.
        for _, _, g in qkv_groups(0, rings=("acc", "stage", "oaug")):
            g()
        fills.extend(qkv_groups(-1))
        if dbg:
            nc.sync.dma_start(dbgb[0:1, 0:1024], QT[0][0][0:1, :])
            nc.sync.dma_start(dbgb[1:2, :], KT[0][0:1, :])
            nc.sync.dma_start(dbgb[2:3, 0:65], VT[0][0][0:1, 0, 0, :])
            nc.sync.dma_start(dbgb[3:4, 0:65], VT[0][0][0:1, 0, 1, :])
        for p in (1, 2, 3):
            fills.extend(qkv_groups(p))

        # ---- seamless pipelined attention over all (pr, qb, tg) ----------
        stagedq = deque()
        oaug_map = {}

        def emit_pv(ent):
            pr, qb, tg, P0, P1, t0, t1 = ent
            if tg == 0:
                o0 = ps.tile([65, 512], F32, tag="oaug", bufs=2, name="oaug0")
                o1 = ps.tile([65, 512], F32, tag="oaug", bufs=2, name="oaug1")
                oaug_map[(pr, qb)] = (o0, o1)
            o0, o1 = oaug_map[(pr, qb)]
            st, sp = tg == 0, tg == 7
            need("v", pr, t0)
            need("v", pr, t1)
            assert ("v", pr, t0) in done and ("v", pr, t1) in done, \
                ("V missing", pr, t0, t1)
            V0, V1 = VT[pr][t0 // 8], VT[pr][t1 // 8]
            nc.tensor.matmul(o0[:], V0[:, t0 % 8, 0, :], P0[:, 0:512],
                             start=st, stop=False)
            nc.tensor.matmul(o0[:], V1[:, t1 % 8, 0, :], P0[:, 512:1024],
                             start=False, stop=sp)
            nc.tensor.matmul(o1[:], V0[:, t0 % 8, 1, :], P1[:, 0:512],
                             start=st, stop=False)
            nc.tensor.matmul(o1[:], V1[:, t1 % 8, 1, :], P1[:, 512:1024],
                             start=False, stop=sp)
            if sp:
                emit_norm(pr, qb, o0, o1)
                del oaug_map[(pr, qb)]

        SLOTS = [(pr, qb, tg) for pr in range(4) for qb in range(4)
                 for tg in range(8)]
        for g, (pr, qb, tg) in enumerate(SLOTS):

            q0 = qb * 512
            t0, t1 = 2 * tg, 2 * tg + 1
            stage0 = ps.tile([128, 1024], F32, tag="stage", bufs=2,
                             name="stage0")
            stage1 = ps.tile([128, 1024], F32, tag="stage", bufs=2,
                             name="stage1")
            # scores S^T [keys, queries]; heads (2pr, 2pr+1) row-packed
            for _nb in range(8):
                need("k", pr, _nb)
                assert ("k", pr, _nb) in done, ("K missing", pr, _nb, g)
            for _nb in range(4 * (qb // 2), 4 * (qb // 2) + 4):
                need("q", pr, _nb)
                assert ("q", pr, _nb) in done, ("Q missing", pr, _nb, g)
            QTh = QT[pr][qb // 2]
            qh0 = (qb % 2) * 512
            nc.tensor.matmul(stage0[:, 0:512],
                             KT[pr][0:64, t0 * 128:(t0 + 1) * 128],
                             QTh[0:64, qh0:qh0 + 512],
                             start=True, stop=True, tile_position=(0, 0))
            nc.tensor.matmul(stage1[:, 0:512],
                             KT[pr][64:128, t0 * 128:(t0 + 1) * 128],
                             QTh[64:128, qh0:qh0 + 512],
                             start=True, stop=True, tile_position=(64, 0))
            nc.tensor.matmul(stage0[:, 512:1024],
                             KT[pr][0:64, t1 * 128:(t1 + 1) * 128],
                             QTh[0:64, qh0:qh0 + 512],
                             start=True, stop=True, tile_position=(0, 0))
            nc.tensor.matmul(stage1[:, 512:1024],
                             KT[pr][64:128, t1 * 128:(t1 + 1) * 128],
                             QTh[64:128, qh0:qh0 + 512],
                             start=True, stop=True, tile_position=(64, 0))
            if len(stagedq) == 2:
                emit_pv(stagedq.popleft())
            P0 = sb.tile([128, 1024], BF16, tag="p", bufs=6, name="P0")
            P1 = sb.tile([128, 1024], BF16, tag="p", bufs=6, name="P1")
            nc.scalar.activation(P0[:], stage0[:], EXP, bias=0.0, scale=SCALE)
            nc.scalar.activation(P1[:], stage1[:], EXP, bias=0.0, scale=SCALE)
            if dbg and g == 0:
                nc.sync.dma_start(dbgb[4:5, 0:1024], P0[0:1, :])
                nc.sync.dma_start(dbgb[5:6, 0:1024], P1[0:1, :])
            stagedq.append((pr, qb, tg, P0, P1, t0, t1))
            if pr == 3 and tg == 6 and qb >= 1:
                fills.extend(proj_groups(qb - 1))
            pace()
            for _ in range(2 if pr == 3 else 1):
                if pending:
                    pending.popleft()()

        while stagedq:
            emit_pv(stagedq.popleft())
        while pending:
            pending.popleft()()
        while fills:
            fills.popleft()[2]()
        for _, _, g2 in proj_groups(3, tail=True):
            g2()
    return nc


def _prepare_in_maps(x, qkv_w, qkv_b, proj_w):
    x = np.asarray(x, dtype=np.float32)
    wb = np.asarray(qkv_w, dtype=np.float32).astype(bfloat16)
    pwb = np.asarray(proj_w, dtype=np.float32).astype(bfloat16)
    qkv_b = np.asarray(qkv_b, dtype=np.float32)
    in_maps = []
    for c in range(8):
        b, g = c % 4, c // 4
        w0 = 512 * g
        in_maps.append({
            "xT": np.ascontiguousarray(x[b].T).astype(bfloat16),
            "wcat": np.ascontiguousarray(np.concatenate(
                [wb[:, w0:w0 + 512],
                 wb[:, 1024 + w0:1024 + w0 + 512],
                 wb[:, 2048 + w0:2048 + w0 + 512]], axis=1)),
            "qb": np.ascontiguousarray(qkv_b[w0:w0 + 512].reshape(4, 128).T),
            "kb": np.ascontiguousarray(
                qkv_b[1024 + w0:1024 + w0 + 512].reshape(4, 128).T),
            "pw": np.ascontiguousarray(pwb[w0:w0 + 512, :]),
        })
    return in_maps


def _gather(parts, qkv_b, proj_w, proj_b):
    const_row = (np.asarray(qkv_b)[2048:].astype(np.float64)
                 @ np.asarray(proj_w).astype(np.float64)
                 + np.asarray(proj_b).astype(np.float64))
    out = np.empty((B, N, C), np.float32)
    for b in range(B):
        out[b] = (np.asarray(parts[b]).astype(np.float64)
                  + np.asarray(parts[b + 4]).astype(np.float64)
                  + const_row).astype(np.float32)
    return out


def kernel(**inputs: np.ndarray) -> np.ndarray:
    x = np.asarray(inputs["x"], dtype=np.float32)
    qkv_w = np.asarray(inputs["qkv_w"], dtype=np.float32)
    qkv_b = np.asarray(inputs["qkv_b"], dtype=np.float32)
    proj_w = np.asarray(inputs["proj_w"], dtype=np.float32)
    proj_b = np.asarray(inputs["proj_b"], dtype=np.float32)

    in_maps = _prepare_in_maps(x, qkv_w, qkv_b, proj_w)
    nc = _build()
    nc.finalize()
    res = run_bass_kernel_spmd(nc, in_maps, list(range(8)))
    parts = [res.results[c]["out"] for c in range(8)]
    return _gather(parts, qkv_b, proj_w, proj_b)


if __name__ == "__main__":
    import tempfile
    import time

    from concourse.bass_utils import compile_bass_kernel

    t0 = time.time()
    nc = _build()
    nc.compile()
    with tempfile.TemporaryDirectory() as td:
        compile_bass_kernel(nc, td, neff_name="k.neff")
    print(f"COMPILE OK ({time.time() - t0:.0f}s)", flush=True)
